# revision 38
# baseline (speedup 1.0000x reference)
"""Trainium2 Bass kernel for nn_AttentionResBlock (windowed causal attention +
sigmoid*tanh gating + two 1x1 convs), SPMD over 8 NeuronCores.

Sharding: data-parallel over (batch, sequence-half): core i handles batch i//2,
rows [h*2048, (h+1)*2048) with h = i%2, plus a 512-row halo (previous window;
zeros + mask flag for h==0). No cross-core communication.

Host prepares both layouts of x (natural [t,c] tiles AND transposed [c,t]
tiles) pre-arranged so every DMA is a plain contiguous burst — no on-device
transposes, no DMA-transpose mode, no 512B-descriptor stalls. The [ones,zeros]
pad columns (softmax-denominator trick) are baked into the natural tiles.

Per-core pipeline (window = 512 queries, kv = 1024 keys):
  scoresT[j,q] = kvT^T @ qT      (PE bf16, softmax scale folded into exp)
  expT = exp(scale*scoresT)      (ACT, PSUM->SBUF, bf16 out)
  causal mask: affine_select fill=0 on diagonal j-chunks; halo flag multiply
  o_unnorm[q, c+2] = sum_j expT[j,q]^T @ [kv | 1 | 0]  (PE; col 256 = denom)
  o = o_unnorm[:, :256] * recip(denom)   (DVE per-partition scalar)
  oT via PE transpose into PSUM; gating reads PSUM directly:
  u = sigmoid(a) * tanh(a)               (2x ACT + 1 DVE mul)
  res/skip[t,d] = u^T @ [Wr|Ws]^T        (PE, fused along N; GpSimd PSUM->SBUF
      bf16 copy), per-window batched DMA out in a pre-tiled layout the host
      unshuffles. Biases are added on the host (they are zero in this model).

The emission is software-pipelined with a one-window lag so the in-order
engine queues run window w+1's attention while ACT/DVE finish window w's
gating/projections. A few identity matmuls at the head of the Tensor queue
warm the HAM clock-gate during the initial DMA shadow.
"""

import numpy as np

B, T, C = 4, 4096, 256
W = 512                # attention window
TCH = T // 2           # rows per core
TH = TCH + W           # with halo
NWIN = TCH // W        # windows per core (4)
NBLK = TH // W         # 512-row blocks per core (5)
NCORES = 8

_CACHE = {}


def _build_program():
    import concourse.bacc as bacc
    import concourse.bass as bass
    import concourse.mybir as mybir
    import concourse.tile as tile
    from concourse.masks import make_identity

    f32 = mybir.dt.float32
    qdt = mybir.dt.bfloat16
    f8 = mybir.dt.float8e4
    f8e = mybir.dt.float8e5
    DR = mybir.MatmulPerfMode.DoubleRow
    ts = bass.ts

    nc = bacc.Bacc("TRN2", target_bir_lowering=False, debug=False)

    xt_d = nc.dram_tensor("xt", [NBLK, 128, 2, W], f8, kind="ExternalInput").ap()
    xn_d = nc.dram_tensor("xn", [NBLK, 128, 4, C + 2], f8, kind="ExternalInput").ap()
    x16_d = nc.dram_tensor(
        "x16", [NWIN, 128, 4, C + 2], qdt, kind="ExternalInput"
    ).ap()
    wc_d = nc.dram_tensor("wc", [128, 2, 2 * C], qdt, kind="ExternalInput").ap()
    rs_d = nc.dram_tensor("rs", [NWIN, 128, 4, 2 * C], qdt, kind="ExternalOutput").ap()

    Exp = mybir.ActivationFunctionType.Exp
    Tanh = mybir.ActivationFunctionType.Tanh
    Sig = mybir.ActivationFunctionType.Sigmoid

    with tile.TileContext(nc) as tc:
        with (
            tc.tile_pool(name="singles", bufs=1) as singles,
            tc.tile_pool(name="xn", bufs=5) as xn_pool,
            tc.tile_pool(name="xt", bufs=5) as xt_pool,
            tc.tile_pool(name="ex", bufs=8) as ex_pool,
            tc.tile_pool(name="on", bufs=6) as on_pool,
            tc.tile_pool(name="g", bufs=6) as g_pool,
            tc.tile_pool(name="outs", bufs=3) as out_pool,
            tc.tile_pool(name="small", bufs=8) as small,
            tc.tile_pool(name="psc", bufs=2, space="PSUM") as sc_pool,
            tc.tile_pool(name="pav", bufs=2, space="PSUM") as avj_pool,
            tc.tile_pool(name="pt", bufs=2, space="PSUM") as pt_pool,
        ):
            wc_sb = singles.tile([128, 2, 2 * C], qdt)
            xn8b = [
                xn_pool.tile([128, 4, C + 2], f8, tag="xn", name=f"xn{i}")
                for i in range(NBLK)
            ]
            xn16b = [
                xn_pool.tile([128, 4, C + 2], qdt, tag="x16", name=f"x16_{i}")
                for i in range(NWIN)
            ]
            xtb = [
                xt_pool.tile([128, 2, W], f8, tag="xt", name=f"xt{i}")
                for i in range(NBLK)
            ]

            # x streams ordered by first use, split across the sync and
            # gpsimd DMA queues so the two startup-critical tiles land in
            # parallel; weights ride the scalar queue. Blocks 3-4 are
            # emitted later (between windows) so the startup loads don't
            # fight them for the HBM bandwidth all 8 cores share.
            identf = singles.tile([128, 128], f32)
            make_identity(nc, identf)
            ident = singles.tile([128, 128], qdt)
            nc.vector.tensor_copy(ident, identf)
            nc.scalar.dma_start(out=wc_sb, in_=wc_d)
            nc.sync.dma_start(out=xtb[1], in_=xt_d[1])
            nc.sync.dma_start(out=xtb[0], in_=xt_d[0])
            nc.sync.dma_start(out=xn8b[0], in_=xn_d[0])
            nc.sync.dma_start(out=xn16b[0], in_=x16_d[0])
            nc.sync.dma_start(out=xn8b[1], in_=xn_d[1])
            nc.sync.dma_start(out=xtb[2], in_=xt_d[2])
            nc.sync.dma_start(out=xn16b[1], in_=x16_d[1])
            nc.sync.dma_start(out=xn8b[2], in_=xn_d[2])

            def load_late(w):
                # block w+2's tiles, emitted between window stages
                blk = w + 2
                if blk <= 4:
                    nc.sync.dma_start(out=xtb[blk], in_=xt_d[blk])
                    nc.sync.dma_start(out=xn16b[blk - 1], in_=x16_d[blk - 1])
                    nc.sync.dma_start(out=xn8b[blk], in_=xn_d[blk])

            # hoist the ACT table load into the DMA shadow (Exp and Tanh
            # share a table set; Sigmoid does NOT — using it thrashes
            # ACT_TABLE_LOADs, so sigmoid comes from the tanh identity)
            actwarm = singles.tile([128, 2], f32)
            nbias = singles.tile([128, 1], f32)
            nc.vector.memset(nbias, -16.0)
            nc.scalar.activation(out=actwarm[:, 0:1], in_=identf[:, 0:1], func=Exp)
            nc.scalar.activation(out=actwarm[:, 1:2], in_=identf[:, 0:1], func=Tanh)
            # HAM clock-gate warm-up: keep the PE busy until the first x
            # tiles land so the duty-cycle governor ramps to full rate
            # before the real QK burst
            pwarm = pt_pool.tile([128, 128], f32, tag="pt")
            for k in range(24):
                nc.tensor.matmul(pwarm, ident, ident, start=(k == 0), stop=(k == 23))

            def attn_stage(w):
                """scores -> exp -> mask -> AV -> normalize -> oT (PSUM)."""
                qt = xtb[w + 1]

                # ---- scoresT[j, q] per j-chunk-PAIR; exp; mask ----
                # fp8 DoubleRow everywhere: the [128, 2, *] cc-chunked tiles
                # are exactly the two-k-tile layout, so each j-chunk is ONE
                # double-pumped matmul over the 256-channel contraction.
                # Scores for a pair of j-chunks share one 2-bank PSUM tile so
                # exp runs as one wide ACT op; exp output is fp8 pairs, which
                # is again the two-k-tile layout for double-pumped AV.
                exps = []  # pair tiles: (0,1),(2,3),(4,5) full q; (6,7) q>=256
                for pr in range(3):
                    psc = sc_pool.tile([128, 2, W], f32, tag="sc")
                    for i in range(2):
                        jc = 2 * pr + i
                        kvt = xtb[w + jc // 4]
                        nc.tensor.matmul(
                            psc[:, i, :],
                            kvt[:, :, ts(jc % 4, 128)],
                            qt,
                            start=True,
                            stop=True,
                            perf_mode=DR,
                        )
                    ex = ex_pool.tile([128, 2, W], f8e, tag="ex")
                    nc.scalar.activation(
                        out=ex, in_=psc, func=Exp, scale=0.0625, bias=nbias
                    )
                    exps.append(ex)
                kvt = xtb[w + 1]
                psc = sc_pool.tile([128, 2, 256], f32, tag="sc")
                for i, jc in enumerate((6, 7)):
                    nc.tensor.matmul(
                        psc[:, i, :],
                        kvt[:, :, ts(jc % 4, 128)],
                        qt[:, :, 256:512],
                        start=True,
                        stop=True,
                        perf_mode=DR,
                    )
                # pair (6,7) always contains the diagonal -> bf16-only path
                ex67 = ex_pool.tile([128, 2, 256], qdt, tag="ex16")
                nc.scalar.activation(
                    out=ex67, in_=psc, func=Exp, scale=0.0625, bias=nbias
                )

                # causal masks. The fp8 pair (4,5) needs none: its DR
                # consumers (qb>=2) sit in the fully-valid region. The
                # diagonal-crossing slices are masked while CONVERTING to
                # bf16 on GpSimd (affine_select with dtype change), giving
                # the bf16 operands for the diagonal matmuls.
                ex45 = ex_pool.tile([128, 2, 256], qdt, tag="ex16")
                nc.gpsimd.affine_select(
                    out=ex45,
                    in_=exps[2][:, :, 0:256],
                    compare_op=mybir.AluOpType.is_ge,
                    fill=0.0,
                    base=0,
                    channel_multiplier=-1,
                    pattern=[[-128, 2], [1, 256]],
                )
                nc.gpsimd.affine_select(
                    out=ex67,
                    in_=ex67,
                    compare_op=mybir.AluOpType.is_ge,
                    fill=0.0,
                    base=0,
                    channel_multiplier=-1,
                    pattern=[[-128, 2], [1, 256]],
                )
                # no halo flag needed: a zero halo gives scores 0 ->
                # exp(-16) = 1.1e-7, which e5m2 flushes to exactly 0

                # ---- AV + denom; normalize; transpose to oT (PSUM) ----
                # Hybrid precision: the softmax here is diagonal-dominated
                # (self-score ||x||^2/16 ~ 16 vs N(0,1) off-diagonal), so the
                # j-chunk holding each q-block's diagonal uses bf16 values
                # (the fp8 weight quantization self-cancels in num/denom),
                # while pure off-diagonal chunk PAIRS — total softmax weight
                # ~1e-4 — run as fp8 double-pumped matmuls.
                pt4 = pt_pool.tile([128, 2, W], qdt, tag="pt")
                ons = []
                ex16 = {4: ex45[:, 0, :], 5: ex45[:, 1, :],
                        6: ex67[:, 0, :], 7: ex67[:, 1, :]}
                for qb in range(4):
                    pav = avj_pool.tile([128, C + 2], f32, tag="av")
                    # off-diagonal fp8-DR pairs: (0,1),(2,3) always; (4,5)
                    # only once the diagonal has moved past it (qb >= 2)
                    ndr = 2 if qb < 2 else 3
                    for k in range(ndr):
                        xn = xn8b[w + (0 if k < 2 else 1)]
                        nc.tensor.matmul(
                            pav,
                            exps[k][:, :, qb * 128 : qb * 128 + 128],
                            xn[:, (2 * k) % 4 : (2 * k) % 4 + 2, :],
                            start=(k == 0),
                            stop=False,
                            perf_mode=DR,
                        )
                    # diagonal-and-after chunks in pure bf16: jc from the
                    # first not-DR-covered chunk through the diagonal qb+4.
                    # The bf16 masked slices cover q in [0,256) for (4,5)
                    # and [256,512) for (6,7); q_off is relative to those.
                    bfjcs = [jc for jc in (4, 5, 6, 7) if 2 * ndr <= jc <= qb + 4]
                    for n, jc in enumerate(bfjcs):
                        q_off = (qb % 2) * 128
                        nc.tensor.matmul(
                            pav,
                            ex16[jc][:, q_off : q_off + 128],
                            xn16b[w][:, jc % 4, :],
                            start=False,
                            stop=(n == len(bfjcs) - 1),
                        )
                    rc = small.tile([128, 1], f32, tag="rc")
                    nc.vector.reciprocal(rc, pav[:, C : C + 1])
                    on = on_pool.tile([128, C], qdt, tag="on")
                    if w == NWIN - 1:
                        # drain window: DVE is the drain bottleneck, ACT idle
                        nc.scalar.activation(
                            out=on,
                            in_=pav[:, 0:C],
                            func=mybir.ActivationFunctionType.Copy,
                            scale=rc,
                        )
                    else:
                        nc.vector.tensor_scalar_mul(on, pav[:, 0:C], rc)
                    ons.append(on)
                for qb in range(4):
                    for cc in range(2):
                        nc.tensor.transpose(
                            pt4[:, cc, ts(qb, 128)], ons[qb][:, ts(cc, 128)], ident
                        )
                return pt4

            def out_stage(w, pt4, last=False):
                """gating -> projections -> store, for window w.

                u = tanh(a) + tanh(a)*tanh(a/2); the 0.5 from
                sigmoid(a) = (1+tanh(a/2))/2 lives in the host-side weights.

                For the last window everything is cut per q-block so the
                gating -> proj -> copy -> store chain pipelines across
                engines instead of serializing into the drain tail.
                """
                rs_win = out_pool.tile([128, 4, 2 * C], qdt, tag="rs")
                halves = [(0, 0), (1, 256)] if last else [(0, 0)]
                gw = W if not last else 256
                us_by_qb = {}
                for hh, q0 in halves:
                    th2 = g_pool.tile([128, 2, gw], qdt, tag="th2")
                    ta = g_pool.tile([128, 2, gw], qdt, tag="ta")
                    src = pt4 if not last else pt4[:, :, q0 : q0 + 256]
                    nc.scalar.activation(out=th2, in_=src, func=Tanh, scale=0.5)
                    nc.scalar.activation(out=ta, in_=src, func=Tanh)
                    nc.vector.tensor_mul(th2, ta, th2)
                    nc.vector.tensor_add(th2, ta, th2)
                    us_by_qb[2 * hh] = us_by_qb[2 * hh + 1] = th2

                def proj(qb, u_t, u_off):
                    psp = avj_pool.tile([128, 2 * C], f32, tag="av")
                    for cc in range(2):
                        nc.tensor.matmul(
                            psp,
                            u_t[:, cc, u_off : u_off + 128],
                            wc_sb[:, cc, :],
                            start=(cc == 0),
                            stop=(cc == 1),
                        )
                    # PSUM->SBUF move (GpSimd/DMA can't touch PSUM): DVE
                    # normally; in the drain windows ACT has slack, so odd
                    # q-blocks ride ACT there
                    if w >= NWIN - 2 and qb % 2 == 1:
                        nc.scalar.activation(
                            out=rs_win[:, qb, :],
                            in_=psp,
                            func=mybir.ActivationFunctionType.Copy,
                        )
                    else:
                        nc.vector.tensor_copy(rs_win[:, qb, :], psp)

                if last:
                    for qb in range(4):
                        proj(qb, us_by_qb[qb], (qb % 2) * 128)
                        q = nc.sync if qb % 2 == 0 else nc.gpsimd
                        q.dma_start(out=rs_d[w, :, qb, :], in_=rs_win[:, qb, :])
                else:
                    u_t = us_by_qb[0]
                    for qb in range(4):
                        proj(qb, u_t, qb * 128)
                    nc.sync.dma_start(out=rs_d[w], in_=rs_win)

            # software pipeline with a one-window lag
            pts = {}
            pts[0] = attn_stage(0)
            load_late(1)
            for w in range(1, NWIN):
                pts[w] = attn_stage(w)
                load_late(w + 1)
                out_stage(w - 1, pts.pop(w - 1))
            out_stage(NWIN - 1, pts.pop(NWIN - 1), last=True)

    nc.compile()
    return nc


def _get_program():
    if "nc" not in _CACHE:
        _CACHE["nc"] = _build_program()
    return _CACHE["nc"]


def _make_in_maps(x, Wr, br, Ws, bs):
    import ml_dtypes

    bf16 = ml_dtypes.bfloat16
    x = np.asarray(x, dtype=np.float32)
    Wr = np.asarray(Wr, dtype=np.float32)
    Ws = np.asarray(Ws, dtype=np.float32)

    # 0.5x from the sigmoid(a) = (1 + tanh(a/2))/2 identity folded into
    # weights; res/skip projections fused along the output dim
    wcomb = 0.5 * np.concatenate([Wr.T, Ws.T], axis=1)    # [C, 2C]
    wc_h = np.ascontiguousarray(
        wcomb.reshape(2, 128, 2 * C).transpose(1, 0, 2)
    ).astype(bf16)

    in_maps = []
    for i in range(NCORES):
        b, h = divmod(i, 2)
        xhf = np.empty((TH, C), np.float32)
        if h == 0:
            xhf[:W] = 0.0
        else:
            xhf[:W] = x[b, TCH - W : TCH]
        xhf[W:] = x[b, h * TCH : (h + 1) * TCH]
        # transposed tiles: xt[blk, p, cc, t] = x[blk*512 + t, cc*128 + p]
        # (fp8 e4m3: QK runs double-pumped; |x| ~ 5 max, far below 240)
        xt_h = np.ascontiguousarray(
            xhf.reshape(NBLK, W, 2, 128).transpose(0, 3, 2, 1)
        ).astype(ml_dtypes.float8_e4m3)
        # natural tiles + baked [ones, zeros] pad columns (denominator trick)
        # fp8 copy for off-diagonal AV pairs, bf16 copy (blocks 1-4) for the
        # diagonal-chunk matmuls
        f8 = ml_dtypes.float8_e4m3
        xn_core = xhf.reshape(NBLK, 4, 128, C).transpose(0, 2, 1, 3)
        xn_h = np.empty((NBLK, 128, 4, C + 2), f8)
        xn_h[:, :, :, 0:C] = xn_core.astype(f8)
        xn_h[:, :, :, C] = f8(1.0)
        xn_h[:, :, :, C + 1] = f8(0.0)
        x16_h = np.empty((NWIN, 128, 4, C + 2), bf16)
        x16_h[:, :, :, 0:C] = xn_core[1:].astype(bf16)
        x16_h[:, :, :, C] = bf16(1.0)
        x16_h[:, :, :, C + 1] = bf16(0.0)
        in_maps.append(
            {
                "xt": xt_h,
                "xn": np.ascontiguousarray(xn_h),
                "x16": np.ascontiguousarray(x16_h),
                "wc": wc_h,
            }
        )
    return in_maps


def _gather(results, br, bs):
    residual = np.empty((B, T, C), np.float32)
    skip = np.empty((B, T, C), np.float32)
    for i in range(NCORES):
        b, h = divmod(i, 2)
        rs = np.asarray(results[i]["rs"], dtype=np.float32)  # [4, 128, 4, 2C]
        rs = rs.transpose(0, 2, 1, 3).reshape(TCH, 2 * C)
        residual[b, h * TCH : (h + 1) * TCH] = rs[:, 0:C]
        skip[b, h * TCH : (h + 1) * TCH] = rs[:, C : 2 * C]
    br = np.asarray(br, dtype=np.float32)
    bs = np.asarray(bs, dtype=np.float32)
    if br.any():
        residual += br
    if bs.any():
        skip += bs
    return residual, skip


def kernel(x, Wr, br, Ws, bs):
    from concourse.bass_utils import run_bass_kernel_spmd

    nc = _get_program()
    in_maps = _make_in_maps(x, Wr, br, Ws, bs)
    res = run_bass_kernel_spmd(nc, in_maps, list(range(NCORES)))
    return _gather(res.results, br, bs)


# revision 39
# speedup vs baseline: 1.0189x; 1.0189x over previous
"""Trainium2 Bass kernel for nn_AttentionResBlock (windowed causal attention +
sigmoid*tanh gating + two 1x1 convs), SPMD over 8 NeuronCores.

Sharding: data-parallel over (batch, sequence-half): core i handles batch i//2,
rows [h*2048, (h+1)*2048) with h = i%2, plus a 512-row halo (previous window;
zeros + mask flag for h==0). No cross-core communication.

Host prepares both layouts of x (natural [t,c] tiles AND transposed [c,t]
tiles) pre-arranged so every DMA is a plain contiguous burst — no on-device
transposes, no DMA-transpose mode, no 512B-descriptor stalls. The [ones,zeros]
pad columns (softmax-denominator trick) are baked into the natural tiles.

Per-core pipeline (window = 512 queries, kv = 1024 keys):
  scoresT[j,q] = kvT^T @ qT      (PE bf16, softmax scale folded into exp)
  expT = exp(scale*scoresT)      (ACT, PSUM->SBUF, bf16 out)
  causal mask: affine_select fill=0 on diagonal j-chunks; halo flag multiply
  o_unnorm[q, c+2] = sum_j expT[j,q]^T @ [kv | 1 | 0]  (PE; col 256 = denom)
  o = o_unnorm[:, :256] * recip(denom)   (DVE per-partition scalar)
  oT via PE transpose into PSUM; gating reads PSUM directly:
  u = sigmoid(a) * tanh(a)               (2x ACT + 1 DVE mul)
  res/skip[t,d] = u^T @ [Wr|Ws]^T        (PE, fused along N; GpSimd PSUM->SBUF
      bf16 copy), per-window batched DMA out in a pre-tiled layout the host
      unshuffles. Biases are added on the host (they are zero in this model).

The emission is software-pipelined with a one-window lag so the in-order
engine queues run window w+1's attention while ACT/DVE finish window w's
gating/projections. A few identity matmuls at the head of the Tensor queue
warm the HAM clock-gate during the initial DMA shadow.
"""

import numpy as np

B, T, C = 4, 4096, 256
W = 512                # attention window
TCH = T // 2           # rows per core
TH = TCH + W           # with halo
NWIN = TCH // W        # windows per core (4)
NBLK = TH // W         # 512-row blocks per core (5)
NCORES = 8

_CACHE = {}


def _build_program():
    import concourse.bacc as bacc
    import concourse.bass as bass
    import concourse.mybir as mybir
    import concourse.tile as tile
    from concourse.masks import make_identity

    f32 = mybir.dt.float32
    qdt = mybir.dt.bfloat16
    f8 = mybir.dt.float8e4
    f8e = mybir.dt.float8e5
    DR = mybir.MatmulPerfMode.DoubleRow
    ts = bass.ts

    nc = bacc.Bacc("TRN2", target_bir_lowering=False, debug=False)

    xt_d = nc.dram_tensor("xt", [NBLK, 128, 2, W], f8, kind="ExternalInput").ap()
    xn_d = nc.dram_tensor("xn", [NBLK, 128, 4, C + 2], f8, kind="ExternalInput").ap()
    x16_d = nc.dram_tensor(
        "x16", [NWIN, 128, 4, C + 2], qdt, kind="ExternalInput"
    ).ap()
    wc_d = nc.dram_tensor("wc", [128, 2, 2 * C], qdt, kind="ExternalInput").ap()
    rs_d = nc.dram_tensor("rs", [NWIN, 128, 4, 2 * C], qdt, kind="ExternalOutput").ap()

    Exp = mybir.ActivationFunctionType.Exp
    Tanh = mybir.ActivationFunctionType.Tanh
    Sig = mybir.ActivationFunctionType.Sigmoid

    with tile.TileContext(nc) as tc:
        with (
            tc.tile_pool(name="singles", bufs=1) as singles,
            tc.tile_pool(name="xn", bufs=5) as xn_pool,
            tc.tile_pool(name="xt", bufs=5) as xt_pool,
            tc.tile_pool(name="ex", bufs=8) as ex_pool,
            tc.tile_pool(name="on", bufs=6) as on_pool,
            tc.tile_pool(name="g", bufs=6) as g_pool,
            tc.tile_pool(name="outs", bufs=3) as out_pool,
            tc.tile_pool(name="small", bufs=8) as small,
            tc.tile_pool(name="psc", bufs=2, space="PSUM") as sc_pool,
            tc.tile_pool(name="pav", bufs=3, space="PSUM") as avj_pool,
            tc.tile_pool(name="pt", bufs=1, space="PSUM") as pt_pool,
        ):
            wc_sb = singles.tile([128, 2, 2 * C], qdt)
            xn8b = [
                xn_pool.tile([128, 4, C + 2], f8, tag="xn", name=f"xn{i}")
                for i in range(NBLK)
            ]
            xn16b = [
                xn_pool.tile([128, 4, C + 2], qdt, tag="x16", name=f"x16_{i}")
                for i in range(NWIN)
            ]
            xtb = [
                xt_pool.tile([128, 2, W], f8, tag="xt", name=f"xt{i}")
                for i in range(NBLK)
            ]

            # x streams ordered by first use, split across the sync and
            # gpsimd DMA queues so the two startup-critical tiles land in
            # parallel; weights ride the scalar queue. Blocks 3-4 are
            # emitted later (between windows) so the startup loads don't
            # fight them for the HBM bandwidth all 8 cores share.
            identf = singles.tile([128, 128], f32)
            make_identity(nc, identf)
            ident = singles.tile([128, 128], qdt)
            nc.vector.tensor_copy(ident, identf)
            nc.scalar.dma_start(out=wc_sb, in_=wc_d)
            nc.sync.dma_start(out=xtb[1], in_=xt_d[1])
            nc.sync.dma_start(out=xtb[0], in_=xt_d[0])
            nc.sync.dma_start(out=xn8b[0], in_=xn_d[0])
            nc.sync.dma_start(out=xn16b[0], in_=x16_d[0])
            nc.sync.dma_start(out=xn8b[1], in_=xn_d[1])
            nc.sync.dma_start(out=xtb[2], in_=xt_d[2])
            nc.sync.dma_start(out=xn16b[1], in_=x16_d[1])
            nc.sync.dma_start(out=xn8b[2], in_=xn_d[2])

            def load_late(w):
                # block w+2's tiles, emitted between window stages
                blk = w + 2
                if blk <= 4:
                    nc.sync.dma_start(out=xtb[blk], in_=xt_d[blk])
                    nc.sync.dma_start(out=xn16b[blk - 1], in_=x16_d[blk - 1])
                    nc.sync.dma_start(out=xn8b[blk], in_=xn_d[blk])

            # hoist the ACT table load into the DMA shadow (Exp and Tanh
            # share a table set; Sigmoid does NOT — using it thrashes
            # ACT_TABLE_LOADs, so sigmoid comes from the tanh identity)
            actwarm = singles.tile([128, 2], f32)
            nbias = singles.tile([128, 1], f32)
            nc.vector.memset(nbias, -16.0)
            nc.scalar.activation(out=actwarm[:, 0:1], in_=identf[:, 0:1], func=Exp)
            nc.scalar.activation(out=actwarm[:, 1:2], in_=identf[:, 0:1], func=Tanh)
            # HAM clock-gate warm-up: keep the PE busy until the first x
            # tiles land so the duty-cycle governor ramps to full rate
            # before the real QK burst
            pwarm = pt_pool.tile([128, 128], f32, tag="pt")
            for k in range(24):
                nc.tensor.matmul(pwarm, ident, ident, start=(k == 0), stop=(k == 23))

            def attn_stage(w):
                """scores -> exp -> mask -> AV -> normalize -> oT (PSUM)."""
                qt = xtb[w + 1]

                # ---- scoresT[j, q] per j-chunk-PAIR; exp; mask ----
                # fp8 DoubleRow everywhere: the [128, 2, *] cc-chunked tiles
                # are exactly the two-k-tile layout, so each j-chunk is ONE
                # double-pumped matmul over the 256-channel contraction.
                # Scores for a pair of j-chunks share one 2-bank PSUM tile so
                # exp runs as one wide ACT op; exp output is fp8 pairs, which
                # is again the two-k-tile layout for double-pumped AV.
                exps = []  # pair tiles: (0,1),(2,3),(4,5) full q; (6,7) q>=256
                for pr in range(3):
                    psc = sc_pool.tile([128, 2, W], f32, tag="sc")
                    for i in range(2):
                        jc = 2 * pr + i
                        kvt = xtb[w + jc // 4]
                        nc.tensor.matmul(
                            psc[:, i, :],
                            kvt[:, :, ts(jc % 4, 128)],
                            qt,
                            start=True,
                            stop=True,
                            perf_mode=DR,
                        )
                    ex = ex_pool.tile([128, 2, W], f8e, tag="ex")
                    nc.scalar.activation(
                        out=ex, in_=psc, func=Exp, scale=0.0625, bias=nbias
                    )
                    exps.append(ex)
                kvt = xtb[w + 1]
                psc = sc_pool.tile([128, 2, 256], f32, tag="sc")
                for i, jc in enumerate((6, 7)):
                    nc.tensor.matmul(
                        psc[:, i, :],
                        kvt[:, :, ts(jc % 4, 128)],
                        qt[:, :, 256:512],
                        start=True,
                        stop=True,
                        perf_mode=DR,
                    )
                # pair (6,7) always contains the diagonal -> bf16-only path
                ex67 = ex_pool.tile([128, 2, 256], qdt, tag="ex16")
                nc.scalar.activation(
                    out=ex67, in_=psc, func=Exp, scale=0.0625, bias=nbias
                )

                # causal masks. The fp8 pair (4,5) needs none: its DR
                # consumers (qb>=2) sit in the fully-valid region. The
                # diagonal-crossing slices are masked while CONVERTING to
                # bf16 on GpSimd (affine_select with dtype change), giving
                # the bf16 operands for the diagonal matmuls.
                ex45 = ex_pool.tile([128, 2, 256], qdt, tag="ex16")
                nc.gpsimd.affine_select(
                    out=ex45,
                    in_=exps[2][:, :, 0:256],
                    compare_op=mybir.AluOpType.is_ge,
                    fill=0.0,
                    base=0,
                    channel_multiplier=-1,
                    pattern=[[-128, 2], [1, 256]],
                )
                nc.gpsimd.affine_select(
                    out=ex67,
                    in_=ex67,
                    compare_op=mybir.AluOpType.is_ge,
                    fill=0.0,
                    base=0,
                    channel_multiplier=-1,
                    pattern=[[-128, 2], [1, 256]],
                )
                # no halo flag needed: a zero halo gives scores 0 ->
                # exp(-16) = 1.1e-7, which e5m2 flushes to exactly 0

                # ---- AV + denom; normalize; transpose to oT (PSUM) ----
                # Hybrid precision: the softmax here is diagonal-dominated
                # (self-score ||x||^2/16 ~ 16 vs N(0,1) off-diagonal), so the
                # j-chunk holding each q-block's diagonal uses bf16 values
                # (the fp8 weight quantization self-cancels in num/denom),
                # while pure off-diagonal chunk PAIRS — total softmax weight
                # ~1e-4 — run as fp8 double-pumped matmuls.
                pt4 = pt_pool.tile([128, 2, W], qdt, tag="pt")
                ons = []
                ex16 = {4: ex45[:, 0, :], 5: ex45[:, 1, :],
                        6: ex67[:, 0, :], 7: ex67[:, 1, :]}
                for qb in range(4):
                    pav = avj_pool.tile([128, C + 2], f32, tag="av")
                    # off-diagonal fp8-DR pairs: (0,1),(2,3) always; (4,5)
                    # only once the diagonal has moved past it (qb >= 2)
                    ndr = 2 if qb < 2 else 3
                    for k in range(ndr):
                        xn = xn8b[w + (0 if k < 2 else 1)]
                        nc.tensor.matmul(
                            pav,
                            exps[k][:, :, qb * 128 : qb * 128 + 128],
                            xn[:, (2 * k) % 4 : (2 * k) % 4 + 2, :],
                            start=(k == 0),
                            stop=False,
                            perf_mode=DR,
                        )
                    # diagonal-and-after chunks in pure bf16: jc from the
                    # first not-DR-covered chunk through the diagonal qb+4.
                    # The bf16 masked slices cover q in [0,256) for (4,5)
                    # and [256,512) for (6,7); q_off is relative to those.
                    bfjcs = [jc for jc in (4, 5, 6, 7) if 2 * ndr <= jc <= qb + 4]
                    for n, jc in enumerate(bfjcs):
                        q_off = (qb % 2) * 128
                        nc.tensor.matmul(
                            pav,
                            ex16[jc][:, q_off : q_off + 128],
                            xn16b[w][:, jc % 4, :],
                            start=False,
                            stop=(n == len(bfjcs) - 1),
                        )
                    rc = small.tile([128, 1], f32, tag="rc")
                    nc.vector.reciprocal(rc, pav[:, C : C + 1])
                    on = on_pool.tile([128, C], qdt, tag="on")
                    if w == NWIN - 1:
                        # drain window: DVE is the drain bottleneck, ACT idle
                        nc.scalar.activation(
                            out=on,
                            in_=pav[:, 0:C],
                            func=mybir.ActivationFunctionType.Copy,
                            scale=rc,
                        )
                    else:
                        nc.vector.tensor_scalar_mul(on, pav[:, 0:C], rc)
                    ons.append(on)
                for qb in range(4):
                    for cc in range(2):
                        nc.tensor.transpose(
                            pt4[:, cc, ts(qb, 128)], ons[qb][:, ts(cc, 128)], ident
                        )
                return pt4

            def out_stage(w, pt4, last=False):
                """gating -> projections -> store, for window w.

                u = tanh(a) + tanh(a)*tanh(a/2); the 0.5 from
                sigmoid(a) = (1+tanh(a/2))/2 lives in the host-side weights.

                For the last window everything is cut per q-block so the
                gating -> proj -> copy -> store chain pipelines across
                engines instead of serializing into the drain tail.
                """
                rs_win = out_pool.tile([128, 4, 2 * C], qdt, tag="rs")
                halves = [(0, 0), (1, 256)] if last else [(0, 0)]
                gw = W if not last else 256
                us_by_qb = {}
                for hh, q0 in halves:
                    th2 = g_pool.tile([128, 2, gw], qdt, tag="th2")
                    ta = g_pool.tile([128, 2, gw], qdt, tag="ta")
                    src = pt4 if not last else pt4[:, :, q0 : q0 + 256]
                    nc.scalar.activation(out=th2, in_=src, func=Tanh, scale=0.5)
                    nc.scalar.activation(out=ta, in_=src, func=Tanh)
                    nc.vector.tensor_mul(th2, ta, th2)
                    nc.vector.tensor_add(th2, ta, th2)
                    us_by_qb[2 * hh] = us_by_qb[2 * hh + 1] = th2

                def proj(qb, u_t, u_off):
                    psp = avj_pool.tile([128, 2 * C], f32, tag="av")
                    for cc in range(2):
                        nc.tensor.matmul(
                            psp,
                            u_t[:, cc, u_off : u_off + 128],
                            wc_sb[:, cc, :],
                            start=(cc == 0),
                            stop=(cc == 1),
                        )
                    # PSUM->SBUF move (GpSimd/DMA can't touch PSUM): DVE
                    # normally; in the drain windows ACT has slack, so odd
                    # q-blocks ride ACT there
                    if w >= NWIN - 2 and qb % 2 == 1:
                        nc.scalar.activation(
                            out=rs_win[:, qb, :],
                            in_=psp,
                            func=mybir.ActivationFunctionType.Copy,
                        )
                    else:
                        nc.vector.tensor_copy(rs_win[:, qb, :], psp)

                if last:
                    for qb in range(4):
                        proj(qb, us_by_qb[qb], (qb % 2) * 128)
                        q = nc.sync if qb % 2 == 0 else nc.gpsimd
                        q.dma_start(out=rs_d[w, :, qb, :], in_=rs_win[:, qb, :])
                else:
                    u_t = us_by_qb[0]
                    for qb in range(4):
                        proj(qb, u_t, qb * 128)
                    nc.sync.dma_start(out=rs_d[w], in_=rs_win)

            # software pipeline with a one-window lag
            pts = {}
            pts[0] = attn_stage(0)
            load_late(1)
            for w in range(1, NWIN):
                pts[w] = attn_stage(w)
                load_late(w + 1)
                out_stage(w - 1, pts.pop(w - 1))
            out_stage(NWIN - 1, pts.pop(NWIN - 1), last=True)

    nc.compile()
    return nc


def _get_program():
    if "nc" not in _CACHE:
        _CACHE["nc"] = _build_program()
    return _CACHE["nc"]


def _make_in_maps(x, Wr, br, Ws, bs):
    import ml_dtypes

    bf16 = ml_dtypes.bfloat16
    x = np.asarray(x, dtype=np.float32)
    Wr = np.asarray(Wr, dtype=np.float32)
    Ws = np.asarray(Ws, dtype=np.float32)

    # 0.5x from the sigmoid(a) = (1 + tanh(a/2))/2 identity folded into
    # weights; res/skip projections fused along the output dim
    wcomb = 0.5 * np.concatenate([Wr.T, Ws.T], axis=1)    # [C, 2C]
    wc_h = np.ascontiguousarray(
        wcomb.reshape(2, 128, 2 * C).transpose(1, 0, 2)
    ).astype(bf16)

    in_maps = []
    for i in range(NCORES):
        b, h = divmod(i, 2)
        xhf = np.empty((TH, C), np.float32)
        if h == 0:
            xhf[:W] = 0.0
        else:
            xhf[:W] = x[b, TCH - W : TCH]
        xhf[W:] = x[b, h * TCH : (h + 1) * TCH]
        # transposed tiles: xt[blk, p, cc, t] = x[blk*512 + t, cc*128 + p]
        # (fp8 e4m3: QK runs double-pumped; |x| ~ 5 max, far below 240)
        xt_h = np.ascontiguousarray(
            xhf.reshape(NBLK, W, 2, 128).transpose(0, 3, 2, 1)
        ).astype(ml_dtypes.float8_e4m3)
        # natural tiles + baked [ones, zeros] pad columns (denominator trick)
        # fp8 copy for off-diagonal AV pairs, bf16 copy (blocks 1-4) for the
        # diagonal-chunk matmuls
        f8 = ml_dtypes.float8_e4m3
        xn_core = xhf.reshape(NBLK, 4, 128, C).transpose(0, 2, 1, 3)
        xn_h = np.empty((NBLK, 128, 4, C + 2), f8)
        xn_h[:, :, :, 0:C] = xn_core.astype(f8)
        xn_h[:, :, :, C] = f8(1.0)
        xn_h[:, :, :, C + 1] = f8(0.0)
        x16_h = np.empty((NWIN, 128, 4, C + 2), bf16)
        x16_h[:, :, :, 0:C] = xn_core[1:].astype(bf16)
        x16_h[:, :, :, C] = bf16(1.0)
        x16_h[:, :, :, C + 1] = bf16(0.0)
        in_maps.append(
            {
                "xt": xt_h,
                "xn": np.ascontiguousarray(xn_h),
                "x16": np.ascontiguousarray(x16_h),
                "wc": wc_h,
            }
        )
    return in_maps


def _gather(results, br, bs):
    residual = np.empty((B, T, C), np.float32)
    skip = np.empty((B, T, C), np.float32)
    for i in range(NCORES):
        b, h = divmod(i, 2)
        rs = np.asarray(results[i]["rs"], dtype=np.float32)  # [4, 128, 4, 2C]
        rs = rs.transpose(0, 2, 1, 3).reshape(TCH, 2 * C)
        residual[b, h * TCH : (h + 1) * TCH] = rs[:, 0:C]
        skip[b, h * TCH : (h + 1) * TCH] = rs[:, C : 2 * C]
    br = np.asarray(br, dtype=np.float32)
    bs = np.asarray(bs, dtype=np.float32)
    if br.any():
        residual += br
    if bs.any():
        skip += bs
    return residual, skip


def kernel(x, Wr, br, Ws, bs):
    from concourse.bass_utils import run_bass_kernel_spmd

    nc = _get_program()
    in_maps = _make_in_maps(x, Wr, br, Ws, bs)
    res = run_bass_kernel_spmd(nc, in_maps, list(range(NCORES)))
    return _gather(res.results, br, bs)
